# revision 6
# baseline (speedup 1.0000x reference)
"""Trainium2 Bass kernel for nn_CustomNetwork (4-layer 4096x4096 MLP with
train-mode BatchNorm1d + ReLU per layer, batch-axis softmax at the end).

Strategy: data-parallel over the batch dim across 8 NeuronCores (512 rows
per core). Activations live in SBUF transposed (channels on partitions,
batch on the free dim) so BatchNorm stats and the batch-axis softmax are
native free-axis reductions. Matmuls run in fp16 (half the weight DMA of
fp32, and the GPIO power throttle pins the PE near 1.95 GHz either way, so
fp16's precision comes free vs bf16). BatchNorm statistics and softmax
sums stay in fp32. Weights are host-retiled to [L, KT, NSUP, P, 512] so
every weight-tile DMA is one 128 KiB contiguous block. PSUM is managed as
eight independent single-bank tiles so accumulation-group dependencies
stay per-bank.

The body is PE-bound and gapless (one MM every 512 PE cycles); the
schedule is built to keep the post-last-matmul tail minimal:
  - The device stores UNNORMALIZED exp values plus per-core local softmax
    sums; the host divides by the (host-reduced) global sums during
    unshard. This removes both softmax-sum collectives from the device
    critical path -- only BN-stat allreduces remain.
  - Mid layers use three stat chunks [0,24)/[24,28)/[28,32) whose
    allreduce round-trips hide under the next layer's matmuls.
  - The last layer chunks stats as [0,16)/[16,20)/[20,24) (each tile is
    exp'ed and stored as soon as its chunk lands), then [24,28) right
    after its supertile (read straight from held PSUM), and runs the
    final supertile tile-major with stats split {28,29,30} / {31} so the
    only collective that trails the last matmul is a 1 KiB allreduce for
    tile 31, followed by one exp + two small stores.
  - All collective staging DMAs and last-layer output stores ride the
    Activation DGE queue, keeping them off the SP queue that streams
    weights.

Note: the Linear bias `b` is mathematically canceled by BatchNorm's mean
subtraction, so it is never loaded.
"""

import numpy as np

import concourse.bacc as bacc
import concourse.mybir as mybir
import concourse.tile as tile
from concourse import bass_utils

P = 128  # SBUF partitions
D = 4096  # feature width
KT = D // P  # 32 k/n tiles
BM = 512  # per-core batch (4096 / 8 cores)
NSUP = 8  # n supertiles of 512 output channels
L = 4  # layers
N_CORES = 8
BN_EPS = 1e-5
# BN-stat allreduce chunks (mid layers).
CHUNKS = [(0, 24), (24, 28), (28, 32)]
# last layer: early chunks finish tiles [0,24) well before the final
# matmuls so their exp+store traffic all hides under the matmul phase
CHUNKS_LAST = [(0, 16), (16, 20), (20, 24)]

F32 = mybir.dt.float32
F16 = mybir.dt.float16

_cached_nc = None


def _bn_scale_shift(nc, small, red, gam_ap, bet_ap, n, tag):
    """From allreduced [P, 2, n] (sum of means, sum of E[h^2]) compute
    scale = gamma/sqrt(var+eps), shift = beta - mean*scale."""
    mean_g = small.tile([P, n], F32, name=f"mean_{tag}")
    var_g = small.tile([P, n], F32, name=f"var_{tag}")
    scale = small.tile([P, n], F32, name=f"scale_{tag}")
    shift = small.tile([P, n], F32, name=f"shift_{tag}")
    nc.vector.tensor_scalar_mul(mean_g[:], red[:, 0, :], 1.0 / N_CORES)
    nc.vector.tensor_scalar_mul(var_g[:], red[:, 1, :], 1.0 / N_CORES)
    # var = E[h^2] - mean^2
    nc.vector.tensor_tensor(scale[:], mean_g[:], mean_g[:], op=mybir.AluOpType.mult)
    nc.vector.tensor_sub(var_g[:], var_g[:], scale[:])
    nc.vector.tensor_scalar_add(var_g[:], var_g[:], BN_EPS)
    nc.scalar.activation(
        scale[:], var_g[:], mybir.ActivationFunctionType.Sqrt, bias=0.0, scale=1.0
    )
    nc.vector.reciprocal(scale[:], scale[:])
    nc.vector.tensor_mul(scale[:], scale[:], gam_ap)
    nc.vector.tensor_tensor(shift[:], mean_g[:], scale[:], op=mybir.AluOpType.mult)
    nc.vector.tensor_sub(shift[:], bet_ap, shift[:])
    return scale, shift


def _pack_stats(nc, small, meanvar, t0, t1, tag):
    """pack[:,0,:] = local mean; pack[:,1,:] = E[h^2] = var + mean^2."""
    n = t1 - t0
    pack = small.tile([P, 2, n], F32, name=f"pack_{tag}")
    nc.vector.tensor_copy(pack[:, 0, :], meanvar[:, t0:t1, 0])
    nc.vector.tensor_tensor(
        pack[:, 1, :], meanvar[:, t0:t1, 0], meanvar[:, t0:t1, 0],
        op=mybir.AluOpType.mult,
    )
    nc.vector.tensor_tensor(
        pack[:, 1, :], pack[:, 1, :], meanvar[:, t0:t1, 1], op=mybir.AluOpType.add
    )
    return pack


def build():
    global _cached_nc
    if _cached_nc is not None:
        return _cached_nc
    nc = bacc.Bacc("TRN2", target_bir_lowering=False, debug=False, num_devices=N_CORES)

    xt = nc.dram_tensor("xt", [D, BM], F16, kind="ExternalInput")
    # host-retiled weights: [l, k, ns] tile is a contiguous [P, 512] block
    Wt = nc.dram_tensor("W", [L, KT, NSUP, P, 512], F16, kind="ExternalInput")
    # gammaH/betaH are host-transposed to [L, P, KT] so the DMA runs with
    # contiguous lines
    gamma = nc.dram_tensor("gammaH", [L, P, KT], F32, kind="ExternalInput")
    beta = nc.dram_tensor("betaH", [L, P, KT], F32, kind="ExternalInput")
    # unnormalized exp(relu(bn(h))) of the last layer, [channels, batch]
    outt = nc.dram_tensor("outt", [D, BM], F16, kind="ExternalOutput")
    # per-core local softmax sums, [P, KT]: channel t*128+p at [p, t]
    sums = nc.dram_tensor("sums", [P, KT], F32, kind="ExternalOutput")

    rg = [list(range(N_CORES))]

    def allreduce(pool_dram, src_ap, dst_tile, tag, eng=None):
        # staging DMAs ride the Activation queue by default so collective
        # inputs/results never wait behind bulk weight traffic on SP.
        eng = eng or nc.scalar
        ar_in = pool_dram.tile(list(src_ap.shape), F32, name=f"arin_{tag}")
        ar_out = pool_dram.tile(list(src_ap.shape), F32, name=f"arout_{tag}")
        eng.dma_start(ar_in[:], src_ap)
        nc.gpsimd.collective_compute(
            "AllReduce",
            mybir.AluOpType.add,
            replica_groups=rg,
            ins=[ar_in.opt()],
            outs=[ar_out.opt()],
        )
        eng.dma_start(dst_tile[:], ar_out[:])

    with tile.TileContext(nc) as tc:
        with (
            tc.tile_pool(name="hbuf", bufs=1) as hpool,
            tc.tile_pool(name="wpool", bufs=48) as wpool,
            tc.tile_pool(name="w7", bufs=1) as w7pool,
            tc.tile_pool(name="psum", bufs=8, space="PSUM") as psum,
            tc.tile_pool(name="small", bufs=2) as small,
            tc.tile_pool(name="gb", bufs=1) as gbpool,
            tc.tile_pool(name="dram", bufs=1, space="DRAM") as dram,
        ):
            h = [
                hpool.tile([P, KT, BM], F16, name="h_a"),
                hpool.tile([P, KT, BM], F16, name="h_b"),
            ]

            # x^T -> h[0] on the Activation DGE queue while the SP queue
            # streams layer-0 weights, so each k's (x, w) pair lands
            # together and the PE can start streaming early.
            w_pre = []
            for k in range(KT):
                nc.scalar.dma_start(h[0][:, k, :], xt.ap()[k * P : (k + 1) * P, :])
                wt = wpool.tile([P, 512], F16, name="wt")
                nc.sync.dma_start(wt[:], Wt.ap()[0, k, 0])
                w_pre.append(wt)

            gam = gbpool.tile([P, L, KT], F32, name="gam")
            bet = gbpool.tile([P, L, KT], F32, name="bet")
            for l in range(L):
                nc.scalar.dma_start(gam[:, l, :], gamma.ap()[l])
                nc.scalar.dma_start(bet[:, l, :], beta.ap()[l])

            sumexp = small.tile([P, KT], F32, name="sumexp")
            # last layer's final supertile weights, preloaded for the
            # tile-major pass
            wt7 = w7pool.tile([P, KT, 512], F16, name="wt7")

            for l in range(L):
                last = l == L - 1
                src = h[l % 2]
                dst = h[(l + 1) % 2]

                stat6 = small.tile([P, KT, 6], F32, name=f"stat6_{l}")
                meanvar = small.tile([P, KT, 2], F32, name=f"meanvar_{l}")

                # ---- matmul phase: out^T[n, m] = sum_k W[k, n] * h^T[k, m]
                ps_hold = {}
                n_seq = NSUP - 1 if last else NSUP
                for ns in range(n_seq):
                    ps = [psum.tile([P, BM], F32, name="ps") for _ in range(4)]
                    for k in range(KT):
                        if l == 0 and ns == 0:
                            wt = w_pre[k]
                        else:
                            wt = wpool.tile([P, 512], F16, name="wt")
                            nc.sync.dma_start(wt[:], Wt.ap()[l, k, ns])
                        for j in range(4):
                            nc.tensor.matmul(
                                ps[j][:],
                                wt[:, j * P : (j + 1) * P],
                                src[:, k, :],
                                start=(k == 0),
                                stop=(k == KT - 1),
                            )
                    if last and ns == 4:
                        # preload ns=7's weights for the tile-major pass
                        for k in range(KT):
                            nc.sync.dma_start(wt7[:, k, :], Wt.ap()[l, k, 7])
                    # On the last layer, tiles 24..27 skip the pre-BN copy:
                    # the exp-apply reads straight from PSUM (nothing needs
                    # those banks afterwards).
                    hold = last and ns == 6
                    for j in range(4):
                        t = ns * 4 + j
                        nc.vector.bn_stats(stat6[:, t, :], ps[j][:])
                        nc.vector.bn_aggr(meanvar[:, t, :], stat6[:, t, :])
                        if not hold:
                            nc.vector.tensor_copy(dst[:, t, :], ps[j][:])
                    if hold:
                        ps_hold[6] = ps
                if last:
                    # ns=7 tile-major: each tile's 32-step k-loop runs to
                    # completion so its BN stats start while the next tile's
                    # matmuls run; only tile 31's stats trail the last matmul.
                    ps = [psum.tile([P, BM], F32, name="ps") for _ in range(4)]
                    for j in range(4):
                        t = 28 + j
                        for k in range(KT):
                            nc.tensor.matmul(
                                ps[j][:],
                                wt7[:, k, j * P : (j + 1) * P],
                                src[:, k, :],
                                start=(k == 0),
                                stop=(k == KT - 1),
                            )
                        nc.vector.bn_stats(stat6[:, t, :], ps[j][:])
                        nc.vector.bn_aggr(meanvar[:, t, :], stat6[:, t, :])
                    ps_hold[7] = ps

                if not last:
                    # ---- BN: chunked cross-core mean / E[h^2] allreduce +
                    # fused scale/shift/relu apply
                    for ci, (t0, t1) in enumerate(CHUNKS):
                        n = t1 - t0
                        tag = f"{l}_{ci}"
                        pack = _pack_stats(nc, small, meanvar, t0, t1, tag)
                        red = small.tile([P, 2, n], F32, name=f"red_{tag}")
                        allreduce(dram, pack[:], red, tag)
                        scale, shift = _bn_scale_shift(
                            nc, small, red, gam[:, l, t0:t1], bet[:, l, t0:t1], n, tag
                        )
                        for i in range(n):
                            t = t0 + i
                            nc.scalar.activation(
                                dst[:, t, :],
                                dst[:, t, :],
                                mybir.ActivationFunctionType.Relu,
                                bias=shift[:, i : i + 1],
                                scale=scale[:, i : i + 1],
                            )
                else:
                    # ---- last layer: BN + exp; store UNNORMALIZED exp and
                    # the local per-channel exp-sums (host does the softmax
                    # divide during unshard).
                    # exp(relu(z)) = max(exp(z), 1); the DVE max also
                    # accumulates the per-channel exp-sum.
                    def exp_store(t, src_ap, scale, shift, i):
                        nc.scalar.activation(
                            dst[:, t, :],
                            src_ap,
                            mybir.ActivationFunctionType.Exp,
                            bias=shift[:, i : i + 1],
                            scale=scale[:, i : i + 1],
                        )
                        nc.vector.tensor_scalar(
                            dst[:, t, :],
                            dst[:, t, :],
                            1.0,
                            0.0,
                            mybir.AluOpType.max,
                            mybir.AluOpType.add,
                            accum_out=sumexp[:, t : t + 1],
                        )
                        nc.scalar.dma_start(
                            outt.ap()[t * P : (t + 1) * P, :], dst[:, t, :]
                        )

                    # tiles [0,24): chunked stat allreduces, all landing
                    # during the matmul phase
                    for ci, (t0, t1) in enumerate(CHUNKS_LAST):
                        n = t1 - t0
                        tag = f"L_{ci}"
                        pack = _pack_stats(nc, small, meanvar, t0, t1, tag)
                        red = small.tile([P, 2, n], F32, name=f"red_{tag}")
                        allreduce(dram, pack[:], red, tag)
                        scale, shift = _bn_scale_shift(
                            nc, small, red, gam[:, l, t0:t1], bet[:, l, t0:t1], n, tag
                        )
                        for i in range(n):
                            exp_store(t0 + i, dst[:, t0 + i, :], scale, shift, i)

                    # tiles [24,28): stats ready one supertile before the
                    # end; allreduce lands pre-last-matmul, exp reads PSUM
                    packC = _pack_stats(nc, small, meanvar, 24, 28, "nsC")
                    redC = small.tile([P, 2, 4], F32, name="red_nsC")
                    allreduce(dram, packC[:], redC, "nsC")
                    scaleC, shiftC = _bn_scale_shift(
                        nc, small, redC, gam[:, l, 24:28], bet[:, l, 24:28], 4, "nsC"
                    )
                    for i in range(4):
                        exp_store(24 + i, ps_hold[6][i][:], scaleC, shiftC, i)

                    # tiles {28,29,30}: stats allreduce issued as soon as
                    # tile 30's k-loop retires -- lands right at the last
                    # matmul. Tile {31} gets its own 1 KiB allreduce; that
                    # round-trip plus one exp is the whole critical tail.
                    packD = _pack_stats(nc, small, meanvar, 28, 31, "nsD")
                    redD = small.tile([P, 2, 3], F32, name="red_nsD")
                    allreduce(dram, packD[:], redD, "nsD")
                    packE = _pack_stats(nc, small, meanvar, 31, 32, "nsE")
                    redE = small.tile([P, 2, 1], F32, name="red_nsE")
                    allreduce(dram, packE[:], redE, "nsE")
                    scaleD, shiftD = _bn_scale_shift(
                        nc, small, redD, gam[:, l, 28:31], bet[:, l, 28:31], 3, "nsD"
                    )
                    for i in range(3):
                        exp_store(28 + i, ps_hold[7][i][:], scaleD, shiftD, i)
                    scaleE, shiftE = _bn_scale_shift(
                        nc, small, redE, gam[:, l, 31:32], bet[:, l, 31:32], 1, "nsE"
                    )
                    exp_store(31, ps_hold[7][3][:], scaleE, shiftE, 0)

                    # local softmax sums out (host reduces across cores)
                    nc.scalar.dma_start(sums.ap()[:, :], sumexp[:])

    nc.compile()
    _cached_nc = nc
    return nc


def make_in_maps(x, W, gamma, beta):
    """Host-side prep: shard x over the batch dim, transpose to [D, BM],
    convert the matmul operands to fp16 (weights also retiled so each
    [P, 512] tile is contiguous), transpose gamma/beta to [L, P, KT]."""
    x = np.asarray(x, dtype=np.float32)
    W = np.asarray(W, dtype=np.float32)
    gamma = np.asarray(gamma, dtype=np.float32)
    beta = np.asarray(beta, dtype=np.float32)
    # W[l, k*P+p, ns*512+c] -> Wtiled[l, k, ns, p, c]
    Wtiled = np.empty((L, KT, NSUP, P, 512), dtype=np.float16)
    Wtiled[...] = np.ascontiguousarray(W).reshape(L, KT, P, NSUP, 512).transpose(
        0, 1, 3, 2, 4
    )
    # [L, D] -> [L, P, KT]: channel (t*128 + p) lands at [l, p, t]
    gammaH = np.ascontiguousarray(gamma.reshape(L, KT, P).transpose(0, 2, 1))
    betaH = np.ascontiguousarray(beta.reshape(L, KT, P).transpose(0, 2, 1))
    in_maps = []
    for c in range(N_CORES):
        xt_c = np.ascontiguousarray(x[c * BM : (c + 1) * BM, :].T.astype(np.float16))
        in_maps.append(
            {"xt": xt_c, "W": Wtiled, "gammaH": gammaH, "betaH": betaH}
        )
    return in_maps


def kernel(x, W, b, gamma, beta):
    """Full (unsharded) inputs -> full [4096, 4096] softmax output."""
    del b  # canceled by BatchNorm mean subtraction
    nc = build()
    in_maps = make_in_maps(x, W, gamma, beta)
    r = bass_utils.run_bass_kernel_spmd(nc, in_maps, core_ids=list(range(N_CORES)))
    # global softmax denominator: sum the per-core local sums on host.
    # sums[p, t] is channel t*128+p -> flatten to [D] in channel order.
    total = np.zeros((P, KT), dtype=np.float32)
    for c in range(N_CORES):
        total += r.results[c]["sums"]
    denom = total.T.reshape(D)  # [KT, P] -> channel t*128+p
    inv = (1.0 / denom).astype(np.float32)
    out = np.empty((N_CORES * BM, D), dtype=np.float32)
    for c in range(N_CORES):
        out[c * BM : (c + 1) * BM, :] = (
            r.results[c]["outt"].T.astype(np.float32) * inv[None, :]
        )
    return out


# revision 12
# speedup vs baseline: 1.0027x; 1.0027x over previous
"""Trainium2 Bass kernel for nn_CustomNetwork (4-layer 4096x4096 MLP with
train-mode BatchNorm1d + ReLU per layer, batch-axis softmax at the end).

Strategy: data-parallel over the batch dim across 8 NeuronCores (512 rows
per core). Activations live in SBUF transposed (channels on partitions,
batch on the free dim) so BatchNorm stats and the batch-axis softmax are
native free-axis reductions. Matmuls run in fp16 (half the weight DMA of
fp32, and the GPIO power throttle pins the PE near 1.95 GHz either way, so
fp16's precision comes free vs bf16). BatchNorm statistics and softmax
sums stay in fp32. Weights are host-retiled to [L, KT, NSUP, P, 512] so
every weight-tile DMA is one 128 KiB contiguous block. PSUM is managed as
eight independent single-bank tiles so accumulation-group dependencies
stay per-bank.

The body is PE-bound and gapless (one MM every 512 PE cycles); the
schedule is built to keep the post-last-matmul tail minimal:
  - The device stores UNNORMALIZED exp values plus per-core local softmax
    sums; the host divides by the (host-reduced) global sums during
    unshard. This removes both softmax-sum collectives from the device
    critical path -- only BN-stat allreduces remain.
  - Mid layers use three stat chunks [0,24)/[24,28)/[28,32) whose
    allreduce round-trips hide under the next layer's matmuls.
  - The last layer chunks stats as [0,16)/[16,20)/[20,24) (each tile is
    exp'ed and stored as soon as its chunk lands), then [24,28) right
    after its supertile (read straight from held PSUM), and runs the
    final supertile tile-major so only tiles {28..31} need a single
    post-matmul collective (each collective costs a flat ~17us of
    CC-stream time regardless of payload, so exactly one trails the
    last matmul), followed by four exps + stores.
  - Collective staging DMAs and the hidden early stores ride the
    Activation DGE queue; the critical tail stores ride the SP queue,
    which is idle once the last weight block is in.

Note: the Linear bias `b` is mathematically canceled by BatchNorm's mean
subtraction, so it is never loaded.
"""

import numpy as np

import concourse.bacc as bacc
import concourse.mybir as mybir
import concourse.tile as tile
from concourse import bass_utils

P = 128  # SBUF partitions
D = 4096  # feature width
KT = D // P  # 32 k/n tiles
BM = 512  # per-core batch (4096 / 8 cores)
NSUP = 8  # n supertiles of 512 output channels
L = 4  # layers
N_CORES = 8
BN_EPS = 1e-5
# BN-stat allreduce chunks (mid layers).
CHUNKS = [(0, 24), (24, 28), (28, 32)]
# last layer: early chunks finish tiles [0,24) well before the final
# matmuls so their exp+store traffic all hides under the matmul phase
CHUNKS_LAST = [(0, 16), (16, 20), (20, 24)]

F32 = mybir.dt.float32
F16 = mybir.dt.float16

_cached_nc = None


def _bn_scale_shift(nc, small, red, gam_ap, bet_ap, n, tag):
    """From allreduced [P, 2, n] (sum of means, sum of E[h^2]) compute
    scale = gamma/sqrt(var+eps), shift = beta - mean*scale."""
    mean_g = small.tile([P, n], F32, name=f"mean_{tag}")
    var_g = small.tile([P, n], F32, name=f"var_{tag}")
    scale = small.tile([P, n], F32, name=f"scale_{tag}")
    shift = small.tile([P, n], F32, name=f"shift_{tag}")
    nc.vector.tensor_scalar_mul(mean_g[:], red[:, 0, :], 1.0 / N_CORES)
    nc.vector.tensor_scalar_mul(var_g[:], red[:, 1, :], 1.0 / N_CORES)
    # var = E[h^2] - mean^2
    nc.vector.tensor_tensor(scale[:], mean_g[:], mean_g[:], op=mybir.AluOpType.mult)
    nc.vector.tensor_sub(var_g[:], var_g[:], scale[:])
    nc.vector.tensor_scalar_add(var_g[:], var_g[:], BN_EPS)
    nc.scalar.activation(
        scale[:], var_g[:], mybir.ActivationFunctionType.Sqrt, bias=0.0, scale=1.0
    )
    nc.vector.reciprocal(scale[:], scale[:])
    nc.vector.tensor_mul(scale[:], scale[:], gam_ap)
    nc.vector.tensor_tensor(shift[:], mean_g[:], scale[:], op=mybir.AluOpType.mult)
    nc.vector.tensor_sub(shift[:], bet_ap, shift[:])
    return scale, shift


def _pack_stats(nc, small, meanvar, t0, t1, tag):
    """pack[:,0,:] = local mean; pack[:,1,:] = E[h^2] = var + mean^2."""
    n = t1 - t0
    pack = small.tile([P, 2, n], F32, name=f"pack_{tag}")
    nc.vector.tensor_copy(pack[:, 0, :], meanvar[:, t0:t1, 0])
    nc.vector.tensor_tensor(
        pack[:, 1, :], meanvar[:, t0:t1, 0], meanvar[:, t0:t1, 0],
        op=mybir.AluOpType.mult,
    )
    nc.vector.tensor_tensor(
        pack[:, 1, :], pack[:, 1, :], meanvar[:, t0:t1, 1], op=mybir.AluOpType.add
    )
    return pack


def build():
    global _cached_nc
    if _cached_nc is not None:
        return _cached_nc
    nc = bacc.Bacc("TRN2", target_bir_lowering=False, debug=False, num_devices=N_CORES)

    xt = nc.dram_tensor("xt", [D, BM], F16, kind="ExternalInput")
    # host-retiled weights: [l, k, ns] tile is a contiguous [P, 512] block
    Wt = nc.dram_tensor("W", [L, KT, NSUP, P, 512], F16, kind="ExternalInput")
    # gammaH/betaH are host-transposed to [L, P, KT] so the DMA runs with
    # contiguous lines
    gamma = nc.dram_tensor("gammaH", [L, P, KT], F32, kind="ExternalInput")
    beta = nc.dram_tensor("betaH", [L, P, KT], F32, kind="ExternalInput")
    # unnormalized exp(relu(bn(h))) of the last layer, [channels, batch]
    outt = nc.dram_tensor("outt", [D, BM], F16, kind="ExternalOutput")
    # per-core local softmax sums, [P, KT]: channel t*128+p at [p, t]
    sums = nc.dram_tensor("sums", [P, KT], F32, kind="ExternalOutput")

    rg = [list(range(N_CORES))]

    def allreduce(pool_dram, src_ap, dst_tile, tag, eng=None):
        # staging DMAs ride the Activation queue by default so collective
        # inputs/results never wait behind bulk weight traffic on SP.
        eng = eng or nc.scalar
        ar_in = pool_dram.tile(list(src_ap.shape), F32, name=f"arin_{tag}")
        ar_out = pool_dram.tile(list(src_ap.shape), F32, name=f"arout_{tag}")
        eng.dma_start(ar_in[:], src_ap)
        nc.gpsimd.collective_compute(
            "AllReduce",
            mybir.AluOpType.add,
            replica_groups=rg,
            ins=[ar_in.opt()],
            outs=[ar_out.opt()],
        )
        eng.dma_start(dst_tile[:], ar_out[:])

    with tile.TileContext(nc) as tc:
        with (
            tc.tile_pool(name="hbuf", bufs=1) as hpool,
            tc.tile_pool(name="wpool", bufs=48) as wpool,
            tc.tile_pool(name="w7", bufs=1) as w7pool,
            tc.tile_pool(name="psum", bufs=8, space="PSUM") as psum,
            tc.tile_pool(name="small", bufs=2) as small,
            tc.tile_pool(name="gb", bufs=1) as gbpool,
            tc.tile_pool(name="dram", bufs=1, space="DRAM") as dram,
        ):
            h = [
                hpool.tile([P, KT, BM], F16, name="h_a"),
                hpool.tile([P, KT, BM], F16, name="h_b"),
            ]

            # x^T -> h[0] on the Activation DGE queue while the SP queue
            # streams layer-0 weights, so each k's (x, w) pair lands
            # together and the PE can start streaming early.
            w_pre = []
            for k in range(KT):
                nc.scalar.dma_start(h[0][:, k, :], xt.ap()[k * P : (k + 1) * P, :])
                wt = wpool.tile([P, 512], F16, name="wt")
                nc.sync.dma_start(wt[:], Wt.ap()[0, k, 0])
                w_pre.append(wt)

            gam = gbpool.tile([P, L, KT], F32, name="gam")
            bet = gbpool.tile([P, L, KT], F32, name="bet")
            for l in range(L):
                nc.scalar.dma_start(gam[:, l, :], gamma.ap()[l])
                nc.scalar.dma_start(bet[:, l, :], beta.ap()[l])

            sumexp = small.tile([P, KT], F32, name="sumexp")
            # last layer's final supertile weights, preloaded for the
            # tile-major pass
            wt7 = w7pool.tile([P, KT, 512], F16, name="wt7")

            for l in range(L):
                last = l == L - 1
                src = h[l % 2]
                dst = h[(l + 1) % 2]

                stat6 = small.tile([P, KT, 6], F32, name=f"stat6_{l}")
                meanvar = small.tile([P, KT, 2], F32, name=f"meanvar_{l}")

                # ---- matmul phase: out^T[n, m] = sum_k W[k, n] * h^T[k, m]
                ps_hold = {}
                n_seq = NSUP - 1 if last else NSUP
                for ns in range(n_seq):
                    ps = [psum.tile([P, BM], F32, name="ps") for _ in range(4)]
                    for k in range(KT):
                        if l == 0 and ns == 0:
                            wt = w_pre[k]
                        else:
                            wt = wpool.tile([P, 512], F16, name="wt")
                            nc.sync.dma_start(wt[:], Wt.ap()[l, k, ns])
                        for j in range(4):
                            nc.tensor.matmul(
                                ps[j][:],
                                wt[:, j * P : (j + 1) * P],
                                src[:, k, :],
                                start=(k == 0),
                                stop=(k == KT - 1),
                            )
                    if last and ns == 4:
                        # preload ns=7's weights for the tile-major pass
                        for k in range(KT):
                            nc.sync.dma_start(wt7[:, k, :], Wt.ap()[l, k, 7])
                    # On the last layer, tiles 24..27 skip the pre-BN copy:
                    # the exp-apply reads straight from PSUM (nothing needs
                    # those banks afterwards).
                    hold = last and ns == 6
                    for j in range(4):
                        t = ns * 4 + j
                        nc.vector.bn_stats(stat6[:, t, :], ps[j][:])
                        nc.vector.bn_aggr(meanvar[:, t, :], stat6[:, t, :])
                        if not hold:
                            nc.vector.tensor_copy(dst[:, t, :], ps[j][:])
                    if hold:
                        ps_hold[6] = ps
                if last:
                    # ns=7 tile-major: each tile's 32-step k-loop runs to
                    # completion so its BN stats start while the next tile's
                    # matmuls run; only tile 31's stats trail the last matmul.
                    ps = [psum.tile([P, BM], F32, name="ps") for _ in range(4)]
                    for j in range(4):
                        t = 28 + j
                        for k in range(KT):
                            nc.tensor.matmul(
                                ps[j][:],
                                wt7[:, k, j * P : (j + 1) * P],
                                src[:, k, :],
                                start=(k == 0),
                                stop=(k == KT - 1),
                            )
                        nc.vector.bn_stats(stat6[:, t, :], ps[j][:])
                        nc.vector.bn_aggr(meanvar[:, t, :], stat6[:, t, :])
                    ps_hold[7] = ps

                if not last:
                    # ---- BN: chunked cross-core mean / E[h^2] allreduce +
                    # fused scale/shift/relu apply
                    for ci, (t0, t1) in enumerate(CHUNKS):
                        n = t1 - t0
                        tag = f"{l}_{ci}"
                        pack = _pack_stats(nc, small, meanvar, t0, t1, tag)
                        red = small.tile([P, 2, n], F32, name=f"red_{tag}")
                        allreduce(dram, pack[:], red, tag)
                        scale, shift = _bn_scale_shift(
                            nc, small, red, gam[:, l, t0:t1], bet[:, l, t0:t1], n, tag
                        )
                        for i in range(n):
                            t = t0 + i
                            nc.scalar.activation(
                                dst[:, t, :],
                                dst[:, t, :],
                                mybir.ActivationFunctionType.Relu,
                                bias=shift[:, i : i + 1],
                                scale=scale[:, i : i + 1],
                            )
                else:
                    # ---- last layer: BN + exp; store UNNORMALIZED exp and
                    # the local per-channel exp-sums (host does the softmax
                    # divide during unshard).
                    # exp(relu(z)) = max(exp(z), 1); the DVE max also
                    # accumulates the per-channel exp-sum.
                    def exp_store(t, src_ap, scale, shift, i, store_eng=nc.scalar):
                        nc.scalar.activation(
                            dst[:, t, :],
                            src_ap,
                            mybir.ActivationFunctionType.Exp,
                            bias=shift[:, i : i + 1],
                            scale=scale[:, i : i + 1],
                        )
                        nc.vector.tensor_scalar(
                            dst[:, t, :],
                            dst[:, t, :],
                            1.0,
                            0.0,
                            mybir.AluOpType.max,
                            mybir.AluOpType.add,
                            accum_out=sumexp[:, t : t + 1],
                        )
                        store_eng.dma_start(
                            outt.ap()[t * P : (t + 1) * P, :], dst[:, t, :]
                        )

                    # tiles [0,24): chunked stat allreduces, all landing
                    # during the matmul phase
                    for ci, (t0, t1) in enumerate(CHUNKS_LAST):
                        n = t1 - t0
                        tag = f"L_{ci}"
                        pack = _pack_stats(nc, small, meanvar, t0, t1, tag)
                        red = small.tile([P, 2, n], F32, name=f"red_{tag}")
                        allreduce(dram, pack[:], red, tag)
                        scale, shift = _bn_scale_shift(
                            nc, small, red, gam[:, l, t0:t1], bet[:, l, t0:t1], n, tag
                        )
                        for i in range(n):
                            exp_store(t0 + i, dst[:, t0 + i, :], scale, shift, i)

                    # tiles [24,28): stats ready one supertile before the
                    # end; allreduce lands pre-last-matmul, exp reads PSUM
                    packC = _pack_stats(nc, small, meanvar, 24, 28, "nsC")
                    redC = small.tile([P, 2, 4], F32, name="red_nsC")
                    allreduce(dram, packC[:], redC, "nsC")
                    scaleC, shiftC = _bn_scale_shift(
                        nc, small, redC, gam[:, l, 24:28], bet[:, l, 24:28], 4, "nsC"
                    )
                    for i in range(4):
                        exp_store(24 + i, ps_hold[6][i][:], scaleC, shiftC, i)

                    # tiles {28..31}: one allreduce triggered the moment
                    # tile 31's stats retire (each collective costs a flat
                    # ~17us of CC-stream time, so one merged op beats two
                    # split ones). This round-trip plus four exps is the
                    # whole critical tail. Stores ride the SP queue, idle
                    # after the last weight DMA.
                    packZ = _pack_stats(nc, small, meanvar, 28, 32, "nsZ")
                    redZ = small.tile([P, 2, 4], F32, name="red_nsZ")
                    allreduce(dram, packZ[:], redZ, "nsZ")
                    scaleZ, shiftZ = _bn_scale_shift(
                        nc, small, redZ, gam[:, l, 28:32], bet[:, l, 28:32], 4, "nsZ"
                    )
                    for i in range(3):
                        exp_store(28 + i, ps_hold[7][i][:], scaleZ, shiftZ, i,
                                  store_eng=nc.sync)
                    # tile 31: sums store goes out between its accum and its
                    # (larger) exp store
                    nc.scalar.activation(
                        dst[:, 31, :],
                        ps_hold[7][3][:],
                        mybir.ActivationFunctionType.Exp,
                        bias=shiftZ[:, 3:4],
                        scale=scaleZ[:, 3:4],
                    )
                    nc.vector.tensor_scalar(
                        dst[:, 31, :],
                        dst[:, 31, :],
                        1.0,
                        0.0,
                        mybir.AluOpType.max,
                        mybir.AluOpType.add,
                        accum_out=sumexp[:, 31:32],
                    )
                    # local softmax sums out (host reduces across cores)
                    nc.sync.dma_start(sums.ap()[:, :], sumexp[:])
                    nc.sync.dma_start(outt.ap()[31 * P : 32 * P, :], dst[:, 31, :])

    nc.compile()
    _cached_nc = nc
    return nc


def make_in_maps(x, W, gamma, beta):
    """Host-side prep: shard x over the batch dim, transpose to [D, BM],
    convert the matmul operands to fp16 (weights also retiled so each
    [P, 512] tile is contiguous), transpose gamma/beta to [L, P, KT]."""
    x = np.asarray(x, dtype=np.float32)
    W = np.asarray(W, dtype=np.float32)
    gamma = np.asarray(gamma, dtype=np.float32)
    beta = np.asarray(beta, dtype=np.float32)
    # W[l, k*P+p, ns*512+c] -> Wtiled[l, k, ns, p, c]
    Wtiled = np.empty((L, KT, NSUP, P, 512), dtype=np.float16)
    Wtiled[...] = np.ascontiguousarray(W).reshape(L, KT, P, NSUP, 512).transpose(
        0, 1, 3, 2, 4
    )
    # [L, D] -> [L, P, KT]: channel (t*128 + p) lands at [l, p, t]
    gammaH = np.ascontiguousarray(gamma.reshape(L, KT, P).transpose(0, 2, 1))
    betaH = np.ascontiguousarray(beta.reshape(L, KT, P).transpose(0, 2, 1))
    in_maps = []
    for c in range(N_CORES):
        xt_c = np.ascontiguousarray(x[c * BM : (c + 1) * BM, :].T.astype(np.float16))
        in_maps.append(
            {"xt": xt_c, "W": Wtiled, "gammaH": gammaH, "betaH": betaH}
        )
    return in_maps


def kernel(x, W, b, gamma, beta):
    """Full (unsharded) inputs -> full [4096, 4096] softmax output."""
    del b  # canceled by BatchNorm mean subtraction
    nc = build()
    in_maps = make_in_maps(x, W, gamma, beta)
    r = bass_utils.run_bass_kernel_spmd(nc, in_maps, core_ids=list(range(N_CORES)))
    # global softmax denominator: sum the per-core local sums on host.
    # sums[p, t] is channel t*128+p -> flatten to [D] in channel order.
    total = np.zeros((P, KT), dtype=np.float32)
    for c in range(N_CORES):
        total += r.results[c]["sums"]
    denom = total.T.reshape(D)  # [KT, P] -> channel t*128+p
    inv = (1.0 / denom).astype(np.float32)
    out = np.empty((N_CORES * BM, D), dtype=np.float32)
    for c in range(N_CORES):
        out[c * BM : (c + 1) * BM, :] = (
            r.results[c]["outt"].T.astype(np.float32) * inv[None, :]
        )
    return out


# revision 16
# speedup vs baseline: 1.0028x; 1.0001x over previous
"""Trainium2 Bass kernel for nn_CustomNetwork (4-layer 4096x4096 MLP with
train-mode BatchNorm1d + ReLU per layer, batch-axis softmax at the end).

Strategy: data-parallel over the batch dim across 8 NeuronCores (512 rows
per core). Activations live in SBUF transposed (channels on partitions,
batch on the free dim) so BatchNorm stats and the batch-axis softmax are
native free-axis reductions. Matmuls run in fp16 (half the weight DMA of
fp32, and the GPIO power throttle pins the PE near 1.95 GHz either way, so
fp16's precision comes free vs bf16). BatchNorm statistics and softmax
sums stay in fp32. Weights are host-retiled to [L, KT, NSUP, P, 512] so
every weight-tile DMA is one 128 KiB contiguous block. PSUM is managed as
eight independent single-bank tiles so accumulation-group dependencies
stay per-bank.

The body is PE-bound and gapless (one MM every 512 PE cycles); the
schedule is built to keep the post-last-matmul tail minimal:
  - The device stores UNNORMALIZED exp values plus per-core local softmax
    sums; the host divides by the (host-reduced) global sums during
    unshard. This removes both softmax-sum collectives from the device
    critical path -- only BN-stat allreduces remain.
  - Mid layers use three stat chunks [0,24)/[24,28)/[28,32) whose
    allreduce round-trips hide under the next layer's matmuls.
  - The last layer chunks stats as [0,16)/[16,20)/[20,24) (each tile is
    exp'ed and stored as soon as its chunk lands), then [24,28) right
    after its supertile (read straight from held PSUM), and runs the
    final supertile tile-major so only tiles {28..31} need a single
    post-matmul collective (each collective costs a flat ~17us of
    CC-stream time regardless of payload, so exactly one trails the
    last matmul), followed by four exps + stores.
  - Collective staging DMAs and the hidden early stores ride the
    Activation DGE queue; the critical tail stores ride the SP queue,
    which is idle once the last weight block is in.

Note: the Linear bias `b` is mathematically canceled by BatchNorm's mean
subtraction, so it is never loaded.
"""

import numpy as np

import concourse.bacc as bacc
import concourse.mybir as mybir
import concourse.tile as tile
from concourse import bass_utils

P = 128  # SBUF partitions
D = 4096  # feature width
KT = D // P  # 32 k/n tiles
BM = 512  # per-core batch (4096 / 8 cores)
NSUP = 8  # n supertiles of 512 output channels
L = 4  # layers
N_CORES = 8
BN_EPS = 1e-5
# BN-stat allreduce chunks (mid layers).
CHUNKS = [(0, 24), (24, 28), (28, 32)]
# last layer: early chunks finish tiles [0,24) well before the final
# matmuls so their exp+store traffic all hides under the matmul phase
CHUNKS_LAST = [(0, 16), (16, 20), (20, 24)]

F32 = mybir.dt.float32
F16 = mybir.dt.float16

_cached_nc = None


def _bn_scale_shift(nc, small, red, gam_ap, bet_ap, n, tag):
    """From allreduced [P, 2, n] (sum of means, sum of E[h^2]) compute
    scale = gamma/sqrt(var+eps), shift = beta - mean*scale."""
    var_g = small.tile([P, n], F32, name=f"var_{tag}")
    scale = small.tile([P, n], F32, name=f"scale_{tag}")
    shift = small.tile([P, n], F32, name=f"shift_{tag}")
    # packs are pre-scaled by 1/N_CORES on each core, so the allreduced
    # values are already the global mean (red[:,0,:]) / E[h^2] (red[:,1,:])
    # var = E[h^2] - mean^2
    nc.vector.tensor_tensor(scale[:], red[:, 0, :], red[:, 0, :], op=mybir.AluOpType.mult)
    nc.vector.tensor_sub(var_g[:], red[:, 1, :], scale[:])
    nc.vector.tensor_scalar_add(var_g[:], var_g[:], BN_EPS)
    nc.scalar.activation(
        scale[:], var_g[:], mybir.ActivationFunctionType.Sqrt, bias=0.0, scale=1.0
    )
    nc.vector.reciprocal(scale[:], scale[:])
    nc.vector.tensor_mul(scale[:], scale[:], gam_ap)
    nc.vector.tensor_tensor(shift[:], red[:, 0, :], scale[:], op=mybir.AluOpType.mult)
    nc.vector.tensor_sub(shift[:], bet_ap, shift[:])
    return scale, shift


def _pack_stats(nc, small, meanvar, t0, t1, tag):
    """pack[:,0,:] = mean/8; pack[:,1,:] = (var + mean^2)/8 -- pre-scaled
    so the 8-core allreduce sum directly yields global mean / E[h^2]."""
    n = t1 - t0
    pack = small.tile([P, 2, n], F32, name=f"pack_{tag}")
    nc.vector.tensor_scalar_mul(pack[:, 0, :], meanvar[:, t0:t1, 0], 1.0 / N_CORES)
    nc.vector.tensor_tensor(
        pack[:, 1, :], meanvar[:, t0:t1, 0], meanvar[:, t0:t1, 0],
        op=mybir.AluOpType.mult,
    )
    nc.vector.tensor_tensor(
        pack[:, 1, :], pack[:, 1, :], meanvar[:, t0:t1, 1], op=mybir.AluOpType.add
    )
    nc.vector.tensor_scalar_mul(pack[:, 1, :], pack[:, 1, :], 1.0 / N_CORES)
    return pack


def build():
    global _cached_nc
    if _cached_nc is not None:
        return _cached_nc
    nc = bacc.Bacc("TRN2", target_bir_lowering=False, debug=False, num_devices=N_CORES)

    xt = nc.dram_tensor("xt", [D, BM], F16, kind="ExternalInput")
    # host-retiled weights: [l, k, ns] tile is a contiguous [P, 512] block
    Wt = nc.dram_tensor("W", [L, KT, NSUP, P, 512], F16, kind="ExternalInput")
    # gammaH/betaH are host-transposed to [L, P, KT] so the DMA runs with
    # contiguous lines
    gamma = nc.dram_tensor("gammaH", [L, P, KT], F32, kind="ExternalInput")
    beta = nc.dram_tensor("betaH", [L, P, KT], F32, kind="ExternalInput")
    # unnormalized exp(relu(bn(h))) of the last layer, [channels, batch]
    outt = nc.dram_tensor("outt", [D, BM], F16, kind="ExternalOutput")
    # per-core local softmax sums, [P, KT]: channel t*128+p at [p, t]
    sums = nc.dram_tensor("sums", [P, KT], F32, kind="ExternalOutput")

    rg = [list(range(N_CORES))]

    def allreduce(pool_dram, src_ap, dst_tile, tag, eng=None):
        # staging DMAs ride the Activation queue by default so collective
        # inputs/results never wait behind bulk weight traffic on SP.
        eng = eng or nc.scalar
        ar_in = pool_dram.tile(list(src_ap.shape), F32, name=f"arin_{tag}")
        ar_out = pool_dram.tile(list(src_ap.shape), F32, name=f"arout_{tag}")
        eng.dma_start(ar_in[:], src_ap)
        nc.gpsimd.collective_compute(
            "AllReduce",
            mybir.AluOpType.add,
            replica_groups=rg,
            ins=[ar_in.opt()],
            outs=[ar_out.opt()],
        )
        eng.dma_start(dst_tile[:], ar_out[:])

    with tile.TileContext(nc) as tc:
        with (
            tc.tile_pool(name="hbuf", bufs=1) as hpool,
            tc.tile_pool(name="wpool", bufs=48) as wpool,
            tc.tile_pool(name="w7", bufs=1) as w7pool,
            tc.tile_pool(name="psum", bufs=8, space="PSUM") as psum,
            tc.tile_pool(name="small", bufs=2) as small,
            tc.tile_pool(name="gb", bufs=1) as gbpool,
            tc.tile_pool(name="dram", bufs=1, space="DRAM") as dram,
        ):
            h = [
                hpool.tile([P, KT, BM], F16, name="h_a"),
                hpool.tile([P, KT, BM], F16, name="h_b"),
            ]

            # x^T -> h[0] on the Activation DGE queue while the SP queue
            # streams layer-0 weights, so each k's (x, w) pair lands
            # together and the PE can start streaming early.
            w_pre = []
            for k in range(KT):
                nc.scalar.dma_start(h[0][:, k, :], xt.ap()[k * P : (k + 1) * P, :])
                wt = wpool.tile([P, 512], F16, name="wt")
                nc.sync.dma_start(wt[:], Wt.ap()[0, k, 0])
                w_pre.append(wt)

            gam = gbpool.tile([P, L, KT], F32, name="gam")
            bet = gbpool.tile([P, L, KT], F32, name="bet")
            for l in range(L):
                nc.scalar.dma_start(gam[:, l, :], gamma.ap()[l])
                nc.scalar.dma_start(bet[:, l, :], beta.ap()[l])

            sumexp = small.tile([P, KT], F32, name="sumexp")
            # last layer's final supertile weights, preloaded for the
            # tile-major pass
            wt7 = w7pool.tile([P, KT, 512], F16, name="wt7")

            for l in range(L):
                last = l == L - 1
                src = h[l % 2]
                dst = h[(l + 1) % 2]

                stat6 = small.tile([P, KT, 6], F32, name=f"stat6_{l}")
                meanvar = small.tile([P, KT, 2], F32, name=f"meanvar_{l}")

                # ---- matmul phase: out^T[n, m] = sum_k W[k, n] * h^T[k, m]
                ps_hold = {}
                n_seq = NSUP - 1 if last else NSUP
                for ns in range(n_seq):
                    ps = [psum.tile([P, BM], F32, name="ps") for _ in range(4)]
                    for k in range(KT):
                        if l == 0 and ns == 0:
                            wt = w_pre[k]
                        else:
                            wt = wpool.tile([P, 512], F16, name="wt")
                            nc.sync.dma_start(wt[:], Wt.ap()[l, k, ns])
                        for j in range(4):
                            nc.tensor.matmul(
                                ps[j][:],
                                wt[:, j * P : (j + 1) * P],
                                src[:, k, :],
                                start=(k == 0),
                                stop=(k == KT - 1),
                            )
                    if last and ns == 4:
                        # preload ns=7's weights for the tile-major pass
                        for k in range(KT):
                            nc.sync.dma_start(wt7[:, k, :], Wt.ap()[l, k, 7])
                    # On the last layer, tiles 24..27 skip the pre-BN copy:
                    # the exp-apply reads straight from PSUM (nothing needs
                    # those banks afterwards).
                    hold = last and ns == 6
                    for j in range(4):
                        t = ns * 4 + j
                        nc.vector.bn_stats(stat6[:, t, :], ps[j][:])
                        nc.vector.bn_aggr(meanvar[:, t, :], stat6[:, t, :])
                        if not hold:
                            nc.vector.tensor_copy(dst[:, t, :], ps[j][:])
                    if hold:
                        ps_hold[6] = ps
                if last:
                    # ns=7 tile-major: each tile's 32-step k-loop runs to
                    # completion so its BN stats start while the next tile's
                    # matmuls run; only tile 31's stats trail the last matmul.
                    ps = [psum.tile([P, BM], F32, name="ps") for _ in range(4)]
                    for j in range(4):
                        t = 28 + j
                        for k in range(KT):
                            nc.tensor.matmul(
                                ps[j][:],
                                wt7[:, k, j * P : (j + 1) * P],
                                src[:, k, :],
                                start=(k == 0),
                                stop=(k == KT - 1),
                            )
                        nc.vector.bn_stats(stat6[:, t, :], ps[j][:])
                        nc.vector.bn_aggr(meanvar[:, t, :], stat6[:, t, :])
                    ps_hold[7] = ps

                if not last:
                    # ---- BN: chunked cross-core mean / E[h^2] allreduce +
                    # fused scale/shift/relu apply
                    for ci, (t0, t1) in enumerate(CHUNKS):
                        n = t1 - t0
                        tag = f"{l}_{ci}"
                        pack = _pack_stats(nc, small, meanvar, t0, t1, tag)
                        red = small.tile([P, 2, n], F32, name=f"red_{tag}")
                        allreduce(dram, pack[:], red, tag)
                        scale, shift = _bn_scale_shift(
                            nc, small, red, gam[:, l, t0:t1], bet[:, l, t0:t1], n, tag
                        )
                        for i in range(n):
                            t = t0 + i
                            nc.scalar.activation(
                                dst[:, t, :],
                                dst[:, t, :],
                                mybir.ActivationFunctionType.Relu,
                                bias=shift[:, i : i + 1],
                                scale=scale[:, i : i + 1],
                            )
                else:
                    # ---- last layer: BN + exp; store UNNORMALIZED exp and
                    # the local per-channel exp-sums (host does the softmax
                    # divide during unshard).
                    # exp(relu(z)) = max(exp(z), 1); the DVE max also
                    # accumulates the per-channel exp-sum.
                    def exp_store(t, src_ap, scale, shift, i, store_eng=nc.scalar):
                        nc.scalar.activation(
                            dst[:, t, :],
                            src_ap,
                            mybir.ActivationFunctionType.Exp,
                            bias=shift[:, i : i + 1],
                            scale=scale[:, i : i + 1],
                        )
                        nc.vector.tensor_scalar(
                            dst[:, t, :],
                            dst[:, t, :],
                            1.0,
                            0.0,
                            mybir.AluOpType.max,
                            mybir.AluOpType.add,
                            accum_out=sumexp[:, t : t + 1],
                        )
                        store_eng.dma_start(
                            outt.ap()[t * P : (t + 1) * P, :], dst[:, t, :]
                        )

                    # tiles [0,24): chunked stat allreduces, all landing
                    # during the matmul phase
                    for ci, (t0, t1) in enumerate(CHUNKS_LAST):
                        n = t1 - t0
                        tag = f"L_{ci}"
                        pack = _pack_stats(nc, small, meanvar, t0, t1, tag)
                        red = small.tile([P, 2, n], F32, name=f"red_{tag}")
                        allreduce(dram, pack[:], red, tag)
                        scale, shift = _bn_scale_shift(
                            nc, small, red, gam[:, l, t0:t1], bet[:, l, t0:t1], n, tag
                        )
                        for i in range(n):
                            exp_store(t0 + i, dst[:, t0 + i, :], scale, shift, i)

                    # tiles [24,28): stats ready one supertile before the
                    # end; allreduce lands pre-last-matmul, exp reads PSUM
                    packC = _pack_stats(nc, small, meanvar, 24, 28, "nsC")
                    redC = small.tile([P, 2, 4], F32, name="red_nsC")
                    allreduce(dram, packC[:], redC, "nsC")
                    scaleC, shiftC = _bn_scale_shift(
                        nc, small, redC, gam[:, l, 24:28], bet[:, l, 24:28], 4, "nsC"
                    )
                    for i in range(4):
                        exp_store(24 + i, ps_hold[6][i][:], scaleC, shiftC, i)

                    # tiles {28..31}: one allreduce triggered the moment
                    # tile 31's stats retire (each collective costs a flat
                    # ~32us doorbell-to-result regardless of payload, so one
                    # merged op beats two split ones). This round-trip plus
                    # four exps is the whole critical tail. All staging and
                    # stores ride the SP queue, idle after the last weight
                    # DMA; tiles 28-30's stats are pre-staged to DRAM while
                    # tile 31's matmuls still run.
                    packZa = _pack_stats(nc, small, meanvar, 28, 31, "nsZa")
                    packZb = _pack_stats(nc, small, meanvar, 31, 32, "nsZb")
                    redZ = small.tile([P, 2, 4], F32, name="red_nsZ")
                    arZ_in = dram.tile([P, 2, 4], F32, name="arin_nsZ")
                    arZ_out = dram.tile([P, 2, 4], F32, name="arout_nsZ")
                    nc.sync.dma_start(arZ_in[:, :, 0:3], packZa[:])
                    nc.sync.dma_start(arZ_in[:, :, 3:4], packZb[:])
                    nc.gpsimd.collective_compute(
                        "AllReduce",
                        mybir.AluOpType.add,
                        replica_groups=rg,
                        ins=[arZ_in.opt()],
                        outs=[arZ_out.opt()],
                    )
                    nc.sync.dma_start(redZ[:], arZ_out[:])
                    scaleZ, shiftZ = _bn_scale_shift(
                        nc, small, redZ, gam[:, l, 28:32], bet[:, l, 28:32], 4, "nsZ"
                    )
                    for i in range(3):
                        exp_store(28 + i, ps_hold[7][i][:], scaleZ, shiftZ, i,
                                  store_eng=nc.sync)
                    # tile 31: sums store goes out between its accum and its
                    # (larger) exp store
                    nc.scalar.activation(
                        dst[:, 31, :],
                        ps_hold[7][3][:],
                        mybir.ActivationFunctionType.Exp,
                        bias=shiftZ[:, 3:4],
                        scale=scaleZ[:, 3:4],
                    )
                    nc.vector.tensor_scalar(
                        dst[:, 31, :],
                        dst[:, 31, :],
                        1.0,
                        0.0,
                        mybir.AluOpType.max,
                        mybir.AluOpType.add,
                        accum_out=sumexp[:, 31:32],
                    )
                    # local softmax sums out (host reduces across cores)
                    nc.sync.dma_start(sums.ap()[:, :], sumexp[:])
                    nc.sync.dma_start(outt.ap()[31 * P : 32 * P, :], dst[:, 31, :])

    nc.compile()
    _cached_nc = nc
    return nc


def make_in_maps(x, W, gamma, beta):
    """Host-side prep: shard x over the batch dim, transpose to [D, BM],
    convert the matmul operands to fp16 (weights also retiled so each
    [P, 512] tile is contiguous), transpose gamma/beta to [L, P, KT]."""
    x = np.asarray(x, dtype=np.float32)
    W = np.asarray(W, dtype=np.float32)
    gamma = np.asarray(gamma, dtype=np.float32)
    beta = np.asarray(beta, dtype=np.float32)
    # W[l, k*P+p, ns*512+c] -> Wtiled[l, k, ns, p, c]
    Wtiled = np.empty((L, KT, NSUP, P, 512), dtype=np.float16)
    Wtiled[...] = np.ascontiguousarray(W).reshape(L, KT, P, NSUP, 512).transpose(
        0, 1, 3, 2, 4
    )
    # [L, D] -> [L, P, KT]: channel (t*128 + p) lands at [l, p, t]
    gammaH = np.ascontiguousarray(gamma.reshape(L, KT, P).transpose(0, 2, 1))
    betaH = np.ascontiguousarray(beta.reshape(L, KT, P).transpose(0, 2, 1))
    in_maps = []
    for c in range(N_CORES):
        xt_c = np.ascontiguousarray(x[c * BM : (c + 1) * BM, :].T.astype(np.float16))
        in_maps.append(
            {"xt": xt_c, "W": Wtiled, "gammaH": gammaH, "betaH": betaH}
        )
    return in_maps


def kernel(x, W, b, gamma, beta):
    """Full (unsharded) inputs -> full [4096, 4096] softmax output."""
    del b  # canceled by BatchNorm mean subtraction
    nc = build()
    in_maps = make_in_maps(x, W, gamma, beta)
    r = bass_utils.run_bass_kernel_spmd(nc, in_maps, core_ids=list(range(N_CORES)))
    # global softmax denominator: sum the per-core local sums on host.
    # sums[p, t] is channel t*128+p -> flatten to [D] in channel order.
    total = np.zeros((P, KT), dtype=np.float32)
    for c in range(N_CORES):
        total += r.results[c]["sums"]
    denom = total.T.reshape(D)  # [KT, P] -> channel t*128+p
    inv = (1.0 / denom).astype(np.float32)
    out = np.empty((N_CORES * BM, D), dtype=np.float32)
    for c in range(N_CORES):
        out[c * BM : (c + 1) * BM, :] = (
            r.results[c]["outt"].T.astype(np.float32) * inv[None, :]
        )
    return out


# revision 21
# speedup vs baseline: 1.0075x; 1.0046x over previous
"""Trainium2 Bass kernel for nn_CustomNetwork (4-layer 4096x4096 MLP with
train-mode BatchNorm1d + ReLU per layer, batch-axis softmax at the end).

Strategy: data-parallel over the batch dim across 8 NeuronCores (512 rows
per core). Activations live in SBUF transposed (channels on partitions,
batch on the free dim) so BatchNorm stats and the batch-axis softmax are
native free-axis reductions. Matmuls run in fp16 (half the weight DMA of
fp32, and the GPIO power throttle pins the PE near 1.95 GHz either way, so
fp16's precision comes free vs bf16). BatchNorm statistics and softmax
sums stay in fp32. Weights are host-retiled to [L, KT, NSUP, P, 512] so
every weight-tile DMA is one 128 KiB contiguous block. PSUM is managed as
eight independent single-bank tiles so accumulation-group dependencies
stay per-bank.

The body is PE-bound and gapless (one MM every 512 PE cycles); the
schedule is built to keep the post-last-matmul tail minimal:
  - The device stores UNNORMALIZED exp values plus per-core local softmax
    sums; the host divides by the (host-reduced) global sums during
    unshard. This removes both softmax-sum collectives from the device
    critical path -- only BN-stat allreduces remain.
  - Mid layers use three stat chunks [0,24)/[24,28)/[28,32) whose
    allreduce round-trips hide under the next layer's matmuls.
  - The last layer chunks stats as [0,16)/[16,20)/[20,24) (each tile is
    exp'ed and stored as soon as its chunk lands), then [24,28) right
    after its supertile (read straight from held PSUM), and runs the
    final supertile tile-major so only tiles {28..31} need a single
    post-matmul collective (each collective costs a flat ~17us of
    CC-stream time regardless of payload, so exactly one trails the
    last matmul), followed by four exps + stores.
  - Collective staging DMAs and the hidden early stores ride the
    Activation DGE queue; the critical tail stores ride the SP queue,
    which is idle once the last weight block is in.

Note: the Linear bias `b` is mathematically canceled by BatchNorm's mean
subtraction, so it is never loaded.
"""

import numpy as np

import concourse.bacc as bacc
import concourse.mybir as mybir
import concourse.tile as tile
from concourse import bass_utils

P = 128  # SBUF partitions
D = 4096  # feature width
KT = D // P  # 32 k/n tiles
BM = 512  # per-core batch (4096 / 8 cores)
NSUP = 8  # n supertiles of 512 output channels
L = 4  # layers
N_CORES = 8
BN_EPS = 1e-5
# BN-stat allreduce chunks (mid layers).
CHUNKS = [(0, 24), (24, 28), (28, 32)]
# last layer: early chunks finish tiles [0,24) well before the final
# matmuls so their exp+store traffic all hides under the matmul phase
CHUNKS_LAST = [(0, 16), (16, 20), (20, 24)]

F32 = mybir.dt.float32
F16 = mybir.dt.float16
USE_ALLGATHER = True

_cached_nc = None


def _bn_scale_shift(nc, small, red, gam_ap, bet_ap, n, tag):
    """From allreduced [P, 2, n] (sum of means, sum of E[h^2]) compute
    scale = gamma/sqrt(var+eps), shift = beta - mean*scale."""
    var_g = small.tile([P, n], F32, name=f"var_{tag}")
    scale = small.tile([P, n], F32, name=f"scale_{tag}")
    shift = small.tile([P, n], F32, name=f"shift_{tag}")
    # packs are pre-scaled by 1/N_CORES on each core, so the allreduced
    # values are already the global mean (red[:,0,:]) / E[h^2] (red[:,1,:])
    # var = E[h^2] - mean^2
    nc.vector.tensor_tensor(scale[:], red[:, 0, :], red[:, 0, :], op=mybir.AluOpType.mult)
    nc.vector.tensor_sub(var_g[:], red[:, 1, :], scale[:])
    nc.vector.tensor_scalar_add(var_g[:], var_g[:], BN_EPS)
    nc.scalar.activation(
        scale[:], var_g[:], mybir.ActivationFunctionType.Sqrt, bias=0.0, scale=1.0
    )
    nc.vector.reciprocal(scale[:], scale[:])
    nc.vector.tensor_mul(scale[:], scale[:], gam_ap)
    nc.vector.tensor_tensor(shift[:], red[:, 0, :], scale[:], op=mybir.AluOpType.mult)
    nc.vector.tensor_sub(shift[:], bet_ap, shift[:])
    return scale, shift


def _pack_stats(nc, small, meanvar, t0, t1, tag):
    """pack[:,0,:] = mean/8; pack[:,1,:] = (var + mean^2)/8 -- pre-scaled
    so the 8-core allreduce sum directly yields global mean / E[h^2]."""
    n = t1 - t0
    pack = small.tile([P, 2, n], F32, name=f"pack_{tag}")
    nc.vector.tensor_scalar_mul(pack[:, 0, :], meanvar[:, t0:t1, 0], 1.0 / N_CORES)
    nc.vector.tensor_tensor(
        pack[:, 1, :], meanvar[:, t0:t1, 0], meanvar[:, t0:t1, 0],
        op=mybir.AluOpType.mult,
    )
    nc.vector.tensor_tensor(
        pack[:, 1, :], pack[:, 1, :], meanvar[:, t0:t1, 1], op=mybir.AluOpType.add
    )
    nc.vector.tensor_scalar_mul(pack[:, 1, :], pack[:, 1, :], 1.0 / N_CORES)
    return pack


def build():
    global _cached_nc
    if _cached_nc is not None:
        return _cached_nc
    nc = bacc.Bacc("TRN2", target_bir_lowering=False, debug=False, num_devices=N_CORES)

    xt = nc.dram_tensor("xt", [D, BM], F16, kind="ExternalInput")
    # host-retiled weights: [l, k, ns] tile is a contiguous [P, 512] block
    Wt = nc.dram_tensor("W", [L, KT, NSUP, P, 512], F16, kind="ExternalInput")
    # gammaH/betaH are host-transposed to [L, P, KT] so the DMA runs with
    # contiguous lines
    gamma = nc.dram_tensor("gammaH", [L, P, KT], F32, kind="ExternalInput")
    beta = nc.dram_tensor("betaH", [L, P, KT], F32, kind="ExternalInput")
    # unnormalized exp(relu(bn(h))) of the last layer, [channels, batch]
    outt = nc.dram_tensor("outt", [D, BM], F16, kind="ExternalOutput")
    # per-core local softmax sums, [P, KT]: channel t*128+p at [p, t]
    sums = nc.dram_tensor("sums", [P, KT], F32, kind="ExternalOutput")

    rg = [list(range(N_CORES))]

    def allreduce(pool_dram, src_ap, dst_tile, tag, eng=None):
        # staging DMAs ride the Activation queue by default so collective
        # inputs/results never wait behind bulk weight traffic on SP.
        eng = eng or nc.scalar
        ar_in = pool_dram.tile(list(src_ap.shape), F32, name=f"arin_{tag}")
        ar_out = pool_dram.tile(list(src_ap.shape), F32, name=f"arout_{tag}")
        eng.dma_start(ar_in[:], src_ap)
        nc.gpsimd.collective_compute(
            "AllReduce",
            mybir.AluOpType.add,
            replica_groups=rg,
            ins=[ar_in.opt()],
            outs=[ar_out.opt()],
        )
        eng.dma_start(dst_tile[:], ar_out[:])

    with tile.TileContext(nc) as tc:
        with (
            tc.tile_pool(name="hbuf", bufs=1) as hpool,
            tc.tile_pool(name="wpool", bufs=48) as wpool,
            tc.tile_pool(name="w7", bufs=1) as w7pool,
            tc.tile_pool(name="psum", bufs=8, space="PSUM") as psum,
            tc.tile_pool(name="small", bufs=2) as small,
            tc.tile_pool(name="gb", bufs=1) as gbpool,
            tc.tile_pool(name="dram", bufs=1, space="DRAM") as dram,
        ):
            h = [
                hpool.tile([P, KT, BM], F16, name="h_a"),
                hpool.tile([P, KT, BM], F16, name="h_b"),
            ]

            # x^T -> h[0] on the Activation DGE queue while the SP queue
            # streams layer-0 weights, so each k's (x, w) pair lands
            # together and the PE can start streaming early.
            w_pre = []
            for k in range(KT):
                nc.scalar.dma_start(h[0][:, k, :], xt.ap()[k * P : (k + 1) * P, :])
                wt = wpool.tile([P, 512], F16, name="wt")
                nc.sync.dma_start(wt[:], Wt.ap()[0, k, 0])
                w_pre.append(wt)

            gam = gbpool.tile([P, L, KT], F32, name="gam")
            bet = gbpool.tile([P, L, KT], F32, name="bet")
            for l in range(L):
                nc.gpsimd.dma_start(gam[:, l, :], gamma.ap()[l])
                nc.gpsimd.dma_start(bet[:, l, :], beta.ap()[l])

            sumexp = small.tile([P, KT], F32, name="sumexp")
            # last layer's final supertile weights, preloaded for the
            # tile-major pass
            wt7 = w7pool.tile([P, KT, 512], F16, name="wt7")

            for l in range(L):
                last = l == L - 1
                src = h[l % 2]
                dst = h[(l + 1) % 2]

                stat6 = small.tile([P, KT, 6], F32, name=f"stat6_{l}")
                meanvar = small.tile([P, KT, 2], F32, name=f"meanvar_{l}")

                # ---- matmul phase: out^T[n, m] = sum_k W[k, n] * h^T[k, m]
                ps_hold = {}
                n_seq = NSUP - 1 if last else NSUP
                for ns in range(n_seq):
                    ps = [psum.tile([P, BM], F32, name="ps") for _ in range(4)]
                    for k in range(KT):
                        if l == 0 and ns == 0:
                            wt = w_pre[k]
                        else:
                            wt = wpool.tile([P, 512], F16, name="wt")
                            nc.sync.dma_start(wt[:], Wt.ap()[l, k, ns])
                        for j in range(4):
                            nc.tensor.matmul(
                                ps[j][:],
                                wt[:, j * P : (j + 1) * P],
                                src[:, k, :],
                                start=(k == 0),
                                stop=(k == KT - 1),
                            )
                    if last and ns == 4:
                        # preload ns=7's weights for the tile-major pass
                        for k in range(KT):
                            nc.sync.dma_start(wt7[:, k, :], Wt.ap()[l, k, 7])
                    # On the last layer, tiles 24..27 skip the pre-BN copy:
                    # the exp-apply reads straight from PSUM (nothing needs
                    # those banks afterwards).
                    hold = last and ns == 6
                    for j in range(4):
                        t = ns * 4 + j
                        nc.vector.bn_stats(stat6[:, t, :], ps[j][:])
                        nc.vector.bn_aggr(meanvar[:, t, :], stat6[:, t, :])
                        if not hold:
                            nc.vector.tensor_copy(dst[:, t, :], ps[j][:])
                    if hold:
                        ps_hold[6] = ps
                if last:
                    # ns=7 tile-major: each tile's 32-step k-loop runs to
                    # completion so its BN stats start while the next tile's
                    # matmuls run; only tile 31's stats trail the last matmul.
                    ps = [psum.tile([P, BM], F32, name="ps") for _ in range(4)]
                    for j in range(4):
                        t = 28 + j
                        for k in range(KT):
                            nc.tensor.matmul(
                                ps[j][:],
                                wt7[:, k, j * P : (j + 1) * P],
                                src[:, k, :],
                                start=(k == 0),
                                stop=(k == KT - 1),
                            )
                        nc.vector.bn_stats(stat6[:, t, :], ps[j][:])
                        nc.vector.bn_aggr(meanvar[:, t, :], stat6[:, t, :])
                    ps_hold[7] = ps

                if not last:
                    # ---- BN: chunked cross-core mean / E[h^2] allreduce +
                    # fused scale/shift/relu apply
                    for ci, (t0, t1) in enumerate(CHUNKS):
                        n = t1 - t0
                        tag = f"{l}_{ci}"
                        pack = _pack_stats(nc, small, meanvar, t0, t1, tag)
                        red = small.tile([P, 2, n], F32, name=f"red_{tag}")
                        allreduce(dram, pack[:], red, tag)
                        scale, shift = _bn_scale_shift(
                            nc, small, red, gam[:, l, t0:t1], bet[:, l, t0:t1], n, tag
                        )
                        for i in range(n):
                            t = t0 + i
                            nc.scalar.activation(
                                dst[:, t, :],
                                dst[:, t, :],
                                mybir.ActivationFunctionType.Relu,
                                bias=shift[:, i : i + 1],
                                scale=scale[:, i : i + 1],
                            )
                else:
                    # ---- last layer: BN + exp; store UNNORMALIZED exp and
                    # the local per-channel exp-sums (host does the softmax
                    # divide during unshard).
                    # exp(relu(z)) = max(exp(z), 1); the DVE max also
                    # accumulates the per-channel exp-sum.
                    def exp_store(t, src_ap, scale, shift, i, store_eng=nc.gpsimd):
                        nc.scalar.activation(
                            dst[:, t, :],
                            src_ap,
                            mybir.ActivationFunctionType.Exp,
                            bias=shift[:, i : i + 1],
                            scale=scale[:, i : i + 1],
                        )
                        nc.vector.tensor_scalar(
                            dst[:, t, :],
                            dst[:, t, :],
                            1.0,
                            0.0,
                            mybir.AluOpType.max,
                            mybir.AluOpType.add,
                            accum_out=sumexp[:, t : t + 1],
                        )
                        store_eng.dma_start(
                            outt.ap()[t * P : (t + 1) * P, :], dst[:, t, :]
                        )

                    # tiles [0,24): chunked stat allreduces, all landing
                    # during the matmul phase
                    for ci, (t0, t1) in enumerate(CHUNKS_LAST):
                        n = t1 - t0
                        tag = f"L_{ci}"
                        pack = _pack_stats(nc, small, meanvar, t0, t1, tag)
                        red = small.tile([P, 2, n], F32, name=f"red_{tag}")
                        allreduce(dram, pack[:], red, tag)
                        scale, shift = _bn_scale_shift(
                            nc, small, red, gam[:, l, t0:t1], bet[:, l, t0:t1], n, tag
                        )
                        for i in range(n):
                            exp_store(t0 + i, dst[:, t0 + i, :], scale, shift, i)

                    # tiles [24,28): stats ready one supertile before the
                    # end; allreduce lands pre-last-matmul, exp reads PSUM
                    packC = _pack_stats(nc, small, meanvar, 24, 28, "nsC")
                    redC = small.tile([P, 2, 4], F32, name="red_nsC")
                    allreduce(dram, packC[:], redC, "nsC")
                    scaleC, shiftC = _bn_scale_shift(
                        nc, small, redC, gam[:, l, 24:28], bet[:, l, 24:28], 4, "nsC"
                    )
                    for i in range(4):
                        exp_store(24 + i, ps_hold[6][i][:], scaleC, shiftC, i)

                    # tiles {28..31}: one allreduce triggered the moment
                    # tile 31's stats retire (each collective costs a flat
                    # ~32us doorbell-to-result regardless of payload, so one
                    # merged op beats two split ones). This round-trip plus
                    # four exps is the whole critical tail. All staging and
                    # stores ride the SP queue, idle after the last weight
                    # DMA; tiles 28-30's stats are pre-staged to DRAM while
                    # tile 31's matmuls still run.
                    packZa = _pack_stats(nc, small, meanvar, 28, 31, "nsZa")
                    packZb = _pack_stats(nc, small, meanvar, 31, 32, "nsZb")
                    redZ = small.tile([P, 2, 4], F32, name="red_nsZ")
                    arZ_in = dram.tile([P, 2, 4], F32, name="arin_nsZ")
                    nc.sync.dma_start(arZ_in[:, :, 0:3], packZa[:])
                    nc.sync.dma_start(arZ_in[:, :, 3:4], packZb[:])
                    if USE_ALLGATHER:
                        # gather the 8 ranks' raw packs and sum locally on
                        # DVE -- tests whether skipping the CC reduce phase
                        # is cheaper
                        arZ_out = dram.tile([N_CORES, P, 2, 4], F32, name="arout_nsZ")
                        nc.gpsimd.collective_compute(
                            "AllGather",
                            mybir.AluOpType.bypass,
                            replica_groups=rg,
                            ins=[arZ_in.opt()],
                            outs=[arZ_out.opt()],
                        )
                        redG = small.tile([P, N_CORES, 2, 4], F32, name="redG_nsZ")
                        for r in range(N_CORES):
                            eng = nc.sync if r % 2 == 0 else nc.scalar
                            eng.dma_start(redG[:, r, :, :], arZ_out[r])
                        nc.vector.tensor_tensor(
                            redZ[:], redG[:, 0, :, :], redG[:, 1, :, :],
                            op=mybir.AluOpType.add,
                        )
                        for r in range(2, N_CORES):
                            nc.vector.tensor_tensor(
                                redZ[:], redZ[:], redG[:, r, :, :],
                                op=mybir.AluOpType.add,
                            )
                    else:
                        arZ_out = dram.tile([P, 2, 4], F32, name="arout_nsZ")
                        nc.gpsimd.collective_compute(
                            "AllReduce",
                            mybir.AluOpType.add,
                            replica_groups=rg,
                            ins=[arZ_in.opt()],
                            outs=[arZ_out.opt()],
                        )
                        nc.sync.dma_start(redZ[:], arZ_out[:])
                    scaleZ, shiftZ = _bn_scale_shift(
                        nc, small, redZ, gam[:, l, 28:32], bet[:, l, 28:32], 4, "nsZ"
                    )
                    for i in range(3):
                        exp_store(28 + i, ps_hold[7][i][:], scaleZ, shiftZ, i,
                                  store_eng=nc.sync)
                    # tile 31: sums store goes out between its accum and its
                    # (larger) exp store
                    nc.scalar.activation(
                        dst[:, 31, :],
                        ps_hold[7][3][:],
                        mybir.ActivationFunctionType.Exp,
                        bias=shiftZ[:, 3:4],
                        scale=scaleZ[:, 3:4],
                    )
                    nc.vector.tensor_scalar(
                        dst[:, 31, :],
                        dst[:, 31, :],
                        1.0,
                        0.0,
                        mybir.AluOpType.max,
                        mybir.AluOpType.add,
                        accum_out=sumexp[:, 31:32],
                    )
                    # local softmax sums out (host reduces across cores)
                    nc.sync.dma_start(sums.ap()[:, :], sumexp[:])
                    nc.sync.dma_start(outt.ap()[31 * P : 32 * P, :], dst[:, 31, :])

    nc.compile()
    _cached_nc = nc
    return nc


def make_in_maps(x, W, gamma, beta):
    """Host-side prep: shard x over the batch dim, transpose to [D, BM],
    convert the matmul operands to fp16 (weights also retiled so each
    [P, 512] tile is contiguous), transpose gamma/beta to [L, P, KT]."""
    x = np.asarray(x, dtype=np.float32)
    W = np.asarray(W, dtype=np.float32)
    gamma = np.asarray(gamma, dtype=np.float32)
    beta = np.asarray(beta, dtype=np.float32)
    # W[l, k*P+p, ns*512+c] -> Wtiled[l, k, ns, p, c]
    Wtiled = np.empty((L, KT, NSUP, P, 512), dtype=np.float16)
    Wtiled[...] = np.ascontiguousarray(W).reshape(L, KT, P, NSUP, 512).transpose(
        0, 1, 3, 2, 4
    )
    # [L, D] -> [L, P, KT]: channel (t*128 + p) lands at [l, p, t]
    gammaH = np.ascontiguousarray(gamma.reshape(L, KT, P).transpose(0, 2, 1))
    betaH = np.ascontiguousarray(beta.reshape(L, KT, P).transpose(0, 2, 1))
    in_maps = []
    for c in range(N_CORES):
        xt_c = np.ascontiguousarray(x[c * BM : (c + 1) * BM, :].T.astype(np.float16))
        in_maps.append(
            {"xt": xt_c, "W": Wtiled, "gammaH": gammaH, "betaH": betaH}
        )
    return in_maps


def kernel(x, W, b, gamma, beta):
    """Full (unsharded) inputs -> full [4096, 4096] softmax output."""
    del b  # canceled by BatchNorm mean subtraction
    nc = build()
    in_maps = make_in_maps(x, W, gamma, beta)
    r = bass_utils.run_bass_kernel_spmd(nc, in_maps, core_ids=list(range(N_CORES)))
    # global softmax denominator: sum the per-core local sums on host.
    # sums[p, t] is channel t*128+p -> flatten to [D] in channel order.
    total = np.zeros((P, KT), dtype=np.float32)
    for c in range(N_CORES):
        total += r.results[c]["sums"]
    denom = total.T.reshape(D)  # [KT, P] -> channel t*128+p
    inv = (1.0 / denom).astype(np.float32)
    out = np.empty((N_CORES * BM, D), dtype=np.float32)
    for c in range(N_CORES):
        out[c * BM : (c + 1) * BM, :] = (
            r.results[c]["outt"].T.astype(np.float32) * inv[None, :]
        )
    return out
